# revision 1
# baseline (speedup 1.0000x reference)
"""Trainium2 Bass kernel for nn_ChannelizedLinearCompression.

Computation (fp32 reference):
    h1      = relu(einsum('bcn,cnh->bch', x, W1) + b1)   # [B, C, H]
    h2      = relu(einsum('bch,chk->bck', h1, W2) + b2)  # [B, C, 10]
    scalars = einsum('bck,ck->bc', h2, W3) + b3          # [B, C]
    out     = relu(scalars @ Wf1 + bf1) @ Wf2 + bf2      # [B, 16]

Sharding: 2 batch groups x 4 channel groups over 8 cores. Each core gets
x^T[c_loc, N, b_loc] (host-transposed so every big DMA is contiguous) and
computes scalars^T[c_loc, b_loc] on device; the tiny final MLP (0.003% of
the FLOPs) runs on host.

Device per-core dataflow (per local channel c):
  stage1: for each K chunk (128 of N=8192): psum[h_chunk][b_half] +=
          W1[k,h_chunk].T @ xT[k, b_half]   (M=h orientation: h1 lands
          h-major so stages 2/3 chain with no transposes, and b1 bias is
          a per-partition ScalarE activation bias)
  stage2: psum2[b_half] += W2[h_chunk].T @ h1T[h_chunk, b_half]; relu+b2
  stage3: psum3[b_half] = W3.T @ h2T[:, b_half]; +b3 -> scalars^T row
"""

import os
from contextlib import ExitStack

import numpy as np

import concourse.bass as bass
import concourse.tile as tile
from concourse import bacc, mybir
from concourse.bass_utils import run_bass_kernel_spmd
from concourse._compat import get_trn_type

# Problem shapes (hardcoded; kernel.py must be self-contained).
B, C, N = 2048, 12, 8192
H, MID = 286, 10
FINAL_HIDDEN, LOWDIM = 30, 16
BG, CG = 2, 4  # batch groups x channel groups = 8 cores
B_LOC, C_LOC = B // BG, C // CG

_DT_NAMES = {
    "float32r": mybir.dt.float32r,
    "float32": mybir.dt.float32,
    "bfloat16": mybir.dt.bfloat16,
    "float16": mybir.dt.float16,
}
# Stage-1 (big GEMM) operand dtype. float16 halves the HBM traffic (the
# bottleneck), streams the PE at full rate, and keeps ~11 mantissa bits —
# x values (|x|<6) and W1 (~0.02) sit comfortably in fp16 range.
DT1 = _DT_NAMES[os.environ.get("KERNEL_DT", "float16")]

F32 = mybir.dt.float32
RELU = mybir.ActivationFunctionType.Relu
IDENT = mybir.ActivationFunctionType.Identity

LAST = {}  # introspection for test.py (exec_time_ns etc.); harness ignores


def build_nc(b_loc=B_LOC, c_loc=C_LOC, n=N, dt1=DT1):
    assert n % 128 == 0 and b_loc % 512 == 0
    nk = n // 128
    nj = b_loc // 512
    hch = [(i * 128, min(128, H - i * 128)) for i in range((H + 127) // 128)]

    # float32r operands must be produced by a rounding compute op (BIR
    # verifier rejects DMA-fed fp32r matmuls), so DRAM I/O stays float32 and
    # a DVE tensor_copy rounds each tile to fp32r on-chip.
    round_fp32r = dt1 == mybir.dt.float32r
    io_dt1 = F32 if round_fp32r else dt1
    # stages 2/3 are tiny; run them in the 2-byte dtype when stage 1 uses one
    dt2 = dt1 if dt1 in (mybir.dt.float16, mybir.dt.bfloat16) else F32

    nc = bacc.Bacc(get_trn_type() or "TRN2", target_bir_lowering=False)
    xt = nc.declare_dram_parameter("xt", [c_loc, n, b_loc], io_dt1,
                                   isOutput=False)
    w1 = nc.declare_dram_parameter("w1", [c_loc, n, H], io_dt1, isOutput=False)
    b1 = nc.declare_dram_parameter("b1", [c_loc, H, 1], F32, isOutput=False)
    w2 = nc.declare_dram_parameter("w2", [c_loc, H, MID], dt2, isOutput=False)
    b2 = nc.declare_dram_parameter("b2", [c_loc, MID, 1], F32, isOutput=False)
    w3 = nc.declare_dram_parameter("w3", [c_loc, MID, 1], dt2, isOutput=False)
    b3 = nc.declare_dram_parameter("b3", [c_loc, 1, 1], F32, isOutput=False)
    out = nc.declare_dram_parameter("out", [c_loc, b_loc], F32, isOutput=True)

    with tile.TileContext(nc) as tc, ExitStack() as ctx:
        xp = ctx.enter_context(tc.tile_pool(name="xp", bufs=6))
        wp = ctx.enter_context(tc.tile_pool(name="wp", bufs=6))
        hp = ctx.enter_context(tc.tile_pool(name="hp", bufs=2 * len(hch)))
        sp = ctx.enter_context(tc.tile_pool(name="sp", bufs=24))
        op = ctx.enter_context(tc.tile_pool(name="op", bufs=4))
        pp = ctx.enter_context(
            tc.tile_pool(name="pp", bufs=8, space=bass.MemorySpace.PSUM)
        )

        for c in range(c_loc):
            b1t = [sp.tile([128, 1], F32, tag="b1t", name=f"b1t{c}_{i}")
                   for i in range(len(hch))]
            w2t = [sp.tile([128, MID], dt2, tag="w2t", name=f"w2t{c}_{i}")
                   for i in range(len(hch))]
            for i, (h0, hs) in enumerate(hch):
                nc.sync.dma_start(b1t[i][:hs, :], b1[c, h0:h0 + hs, :])
                nc.sync.dma_start(w2t[i][:hs, :], w2[c, h0:h0 + hs, :])
            w3t = sp.tile([MID, 1], dt2, tag="w3t", name=f"w3t{c}")
            b2t = sp.tile([MID, 1], F32, tag="b2t", name=f"b2t{c}")
            b3t = sp.tile([1, 1], F32, tag="b3t", name=f"b3t{c}")
            nc.sync.dma_start(w3t[:, :], w3[c])
            nc.sync.dma_start(b2t[:, :], b2[c])
            nc.sync.dma_start(b3t[:, :], b3[c])

            # stage 1: h1T[h, b] = relu(W1[c].T @ xT[c] + b1[c])
            ps = [[pp.tile([128, 512], F32, tag="ps", name=f"ps{c}_{i}_{j}")
                   for j in range(nj)] for i in range(len(hch))]
            for k in range(nk):
                xtt = xp.tile([128, b_loc], io_dt1, tag="xtt", name=f"xtt{c}_{k}")
                w1t = wp.tile([128, H], io_dt1, tag="w1t", name=f"w1t{c}_{k}")
                nc.sync.dma_start(xtt[:, :], xt[c, k * 128:(k + 1) * 128, :])
                nc.sync.dma_start(w1t[:, :], w1[c, k * 128:(k + 1) * 128, :])
                if round_fp32r:
                    xttr = xp.tile([128, b_loc], dt1, tag="xttr",
                                   name=f"xttr{c}_{k}")
                    w1tr = wp.tile([128, H], dt1, tag="w1tr",
                                   name=f"w1tr{c}_{k}")
                    nc.vector.tensor_copy(xttr[:, :], xtt[:, :])
                    nc.vector.tensor_copy(w1tr[:, :], w1t[:, :])
                    xtt, w1t = xttr, w1tr
                for i, (h0, hs) in enumerate(hch):
                    for j in range(nj):
                        nc.tensor.matmul(
                            ps[i][j][:hs, :],
                            w1t[:, h0:h0 + hs],
                            xtt[:, j * 512:(j + 1) * 512],
                            start=(k == 0),
                            stop=(k == nk - 1),
                        )
            h1t = [hp.tile([128, b_loc], dt2, tag="h1t", name=f"h1t{c}_{i}")
                   for i in range(len(hch))]
            for i, (h0, hs) in enumerate(hch):
                for j in range(nj):
                    nc.scalar.activation(
                        h1t[i][:hs, j * 512:(j + 1) * 512],
                        ps[i][j][:hs, :],
                        RELU,
                        bias=b1t[i][:hs, :],
                    )

            # stage 2: h2T[k, b] = relu(W2[c].T @ h1T + b2[c])
            p2 = [pp.tile([MID, 512], F32, tag="ps", name=f"p2{c}_{j}")
                  for j in range(nj)]
            for i, (h0, hs) in enumerate(hch):
                for j in range(nj):
                    nc.tensor.matmul(
                        p2[j][:, :],
                        w2t[i][:hs, :],
                        h1t[i][:hs, j * 512:(j + 1) * 512],
                        start=(i == 0),
                        stop=(i == len(hch) - 1),
                    )
            h2t = op.tile([MID, b_loc], dt2, tag="h2t", name=f"h2t{c}")
            for j in range(nj):
                nc.scalar.activation(
                    h2t[:, j * 512:(j + 1) * 512], p2[j][:, :], RELU,
                    bias=b2t[:, :],
                )

            # stage 3: scalarsT[c, b] = W3[c].T @ h2T + b3[c]
            p3 = [pp.tile([1, 512], F32, tag="ps", name=f"p3{c}_{j}")
                  for j in range(nj)]
            sct = op.tile([1, b_loc], F32, tag="sct", name=f"sct{c}")
            for j in range(nj):
                nc.tensor.matmul(
                    p3[j][:, :], w3t[:, :], h2t[:, j * 512:(j + 1) * 512],
                    start=True, stop=True,
                )
                nc.scalar.activation(
                    sct[:, j * 512:(j + 1) * 512], p3[j][:, :], IDENT,
                    bias=b3t[:, :],
                )
            nc.sync.dma_start(out[c:c + 1, :], sct[0:1, :])

    nc.compile()
    return nc


_NC_CACHE = {}


def _get_nc():
    key = (B_LOC, C_LOC, N, DT1)
    if key not in _NC_CACHE:
        _NC_CACHE[key] = build_nc()
    return _NC_CACHE[key]


def _to_dt1(arr):
    """Cast a float32 ndarray to DT1's numpy representation."""
    if DT1 == mybir.dt.bfloat16:
        import ml_dtypes
        try:
            import torch
            t = torch.from_numpy(np.ascontiguousarray(arr))
            return t.to(torch.bfloat16).view(torch.uint16).numpy().view(
                ml_dtypes.bfloat16)
        except ImportError:
            return arr.astype(ml_dtypes.bfloat16)
    if DT1 == mybir.dt.float16:
        return np.ascontiguousarray(arr, dtype=np.float16)
    return np.ascontiguousarray(arr, dtype=np.float32)


def _to_dt2(arr):
    if DT1 in (mybir.dt.float16, mybir.dt.bfloat16):
        return _to_dt1(arr)
    return np.ascontiguousarray(arr, dtype=np.float32)


def _transpose_shard(xs):
    """[b_loc, c_loc, n] -> contiguous [c_loc, n, b_loc]."""
    try:
        import torch
        try:
            torch.set_num_threads(max(os.cpu_count() or 1, 1))
        except Exception:
            pass
        return torch.from_numpy(np.ascontiguousarray(xs)).permute(
            1, 2, 0).contiguous().numpy()
    except ImportError:
        return np.ascontiguousarray(np.transpose(xs, (1, 2, 0)))


def kernel(x, W1, b1, W2, b2, W3, b3, Wf1, bf1, Wf2, bf2):
    x = np.asarray(x, dtype=np.float32)
    W1 = np.asarray(W1, dtype=np.float32)
    b1 = np.asarray(b1, dtype=np.float32)
    W2 = np.asarray(W2, dtype=np.float32)
    b2 = np.asarray(b2, dtype=np.float32)
    W3 = np.asarray(W3, dtype=np.float32)
    b3 = np.asarray(b3, dtype=np.float32)

    nc = _get_nc()

    if DT1 == mybir.dt.float16:
        # cast before transposing so the shuffle moves half the bytes
        x = _to_dt1(x)

    in_maps = []
    for ib in range(BG):
        bs = slice(ib * B_LOC, (ib + 1) * B_LOC)
        for ic in range(CG):
            cs = slice(ic * C_LOC, (ic + 1) * C_LOC)
            in_maps.append({
                "xt": _to_dt1(_transpose_shard(x[bs, cs, :])),
                "w1": _to_dt1(W1[cs]),
                "b1": np.ascontiguousarray(b1[cs])[:, :, None],
                "w2": _to_dt2(W2[cs]),
                "b2": np.ascontiguousarray(b2[cs])[:, :, None],
                "w3": _to_dt2(W3[cs])[:, :, None],
                "b3": np.ascontiguousarray(b3[cs])[:, None, None],
            })

    res = run_bass_kernel_spmd(nc, in_maps, list(range(BG * CG)))
    LAST["exec_time_ns"] = res.exec_time_ns
    LAST["results"] = res

    scalars = np.empty((B, C), np.float32)
    idx = 0
    for ib in range(BG):
        bs = slice(ib * B_LOC, (ib + 1) * B_LOC)
        for ic in range(CG):
            cs = slice(ic * C_LOC, (ic + 1) * C_LOC)
            scalars[bs, cs] = res.results[idx]["out"].T
            idx += 1

    # Final tiny MLP (C -> 30 -> lowdim) on host in fp32.
    h = np.maximum(scalars @ np.asarray(Wf1, np.float32)
                   + np.asarray(bf1, np.float32), 0.0)
    return (h @ np.asarray(Wf2, np.float32)
            + np.asarray(bf2, np.float32)).astype(np.float32)



# revision 3
# speedup vs baseline: 1.2757x; 1.2757x over previous
"""Trainium2 Bass kernel for nn_ChannelizedLinearCompression.

Computation (fp32 reference):
    h1      = relu(einsum('bcn,cnh->bch', x, W1) + b1)   # [B, C, H]
    h2      = relu(einsum('bch,chk->bck', h1, W2) + b2)  # [B, C, 10]
    scalars = einsum('bck,ck->bc', h2, W3) + b3          # [B, C]
    out     = relu(scalars @ Wf1 + bf1) @ Wf2 + bf2      # [B, 16]

Sharding: 2 batch groups x 4 channel groups over 8 cores; the tiny final
MLP (0.003% of FLOPs) runs on host.

Stage 1 runs in batch-orientation: psum[bt][b, h] += xT[k, bt].T @ W1[k, :]
with M=128 batch rows and H=286 as the streamed free dim, so every matmul
uses the full PE array (the h-major layout wastes 77% of the array on the
286 = 128+128+30 remainder chunk). h1 is then PE-transposed back to
h-major (cheap: 128-col streams) so stages 2/3 chain as before and the b1
bias can ride the per-partition ScalarE activation bias.

DMA: x streams as [128, 2048B] lines at full rate on the sync HWDGE
queue. W1 is host-packed 4 k-chunks wide ([128, 2288B] lines — raw
572B/row lines run at ~1/4 DMA rate) and issued on the scalar HWDGE
queue so descriptor generation doesn't serialize behind x.
"""

import numpy as np

from contextlib import ExitStack

import concourse.bass as bass
import concourse.tile as tile
from concourse import bacc, mybir
from concourse.bass_utils import run_bass_kernel_spmd
from concourse.masks import make_identity
from concourse._compat import get_trn_type

# Problem shapes (hardcoded; kernel.py must be self-contained).
B, C, N = 2048, 12, 8192
H, MID = 286, 10
FINAL_HIDDEN, LOWDIM = 30, 16
BG, CG = 2, 4  # batch groups x channel groups = 8 cores
B_LOC, C_LOC = B // BG, C // CG
NK = N // 128            # 64 contraction chunks
KQ = 4                   # k-chunks packed per W1 DMA (2288B lines)
NKQ = NK // KQ
NBT = B_LOC // 128       # 8 batch tiles
NJ = B_LOC // 512        # 2 psum-width groups for stages 2/3
HCH = [(0, 128), (128, 128), (256, 30)]

F16 = mybir.dt.float16
F32 = mybir.dt.float32
RELU = mybir.ActivationFunctionType.Relu
IDENT = mybir.ActivationFunctionType.Identity

LAST = {}  # introspection for test.py (exec_time_ns etc.); harness ignores


def build_nc():
    nc = bacc.Bacc(get_trn_type() or "TRN2", target_bir_lowering=False)
    xt = nc.declare_dram_parameter("xt", [C_LOC, N, B_LOC], F16,
                                   isOutput=False)
    w1p = nc.declare_dram_parameter("w1p", [C_LOC, NKQ, 128, KQ * H], F16,
                                    isOutput=False)
    b1 = nc.declare_dram_parameter("b1", [C_LOC, H, 1], F32, isOutput=False)
    w2 = nc.declare_dram_parameter("w2", [C_LOC, H, MID], F16, isOutput=False)
    b2 = nc.declare_dram_parameter("b2", [C_LOC, MID, 1], F32, isOutput=False)
    w3 = nc.declare_dram_parameter("w3", [C_LOC, MID, 1], F16, isOutput=False)
    b3 = nc.declare_dram_parameter("b3", [C_LOC, 1, 1], F32, isOutput=False)
    out = nc.declare_dram_parameter("out", [C_LOC, B_LOC], F32, isOutput=True)

    with tile.TileContext(nc) as tc, ExitStack() as ctx:
        xp = ctx.enter_context(tc.tile_pool(name="xp", bufs=8))
        wp = ctx.enter_context(tc.tile_pool(name="wp", bufs=3))
        hp = ctx.enter_context(tc.tile_pool(name="hp", bufs=6))
        h1p = ctx.enter_context(tc.tile_pool(name="h1p", bufs=3 * C_LOC))
        sp = ctx.enter_context(tc.tile_pool(name="sp", bufs=1))
        op = ctx.enter_context(tc.tile_pool(name="op", bufs=3))
        pp = ctx.enter_context(
            tc.tile_pool(name="pp", bufs=8, space=bass.MemorySpace.PSUM)
        )

        ident = sp.tile([128, 128], F16, tag="ident", name="ident")
        make_identity(nc, ident[:, :])

        # All small per-channel weights up front (scalar HWDGE queue).
        b1t, w2t, w3t, b2t, b3t = {}, {}, {}, {}, {}
        for c in range(C_LOC):
            for i, (h0, hs) in enumerate(HCH):
                b1t[c, i] = sp.tile([128, 1], F32, tag=f"b1_{c}_{i}",
                                    name=f"b1_{c}_{i}")
                w2t[c, i] = sp.tile([128, MID], F16, tag=f"w2_{c}_{i}",
                                    name=f"w2_{c}_{i}")
                nc.scalar.dma_start(b1t[c, i][:hs, :], b1[c, h0:h0 + hs, :])
                nc.scalar.dma_start(w2t[c, i][:hs, :], w2[c, h0:h0 + hs, :])
            w3t[c] = sp.tile([MID, 1], F16, tag=f"w3_{c}", name=f"w3_{c}")
            b2t[c] = sp.tile([MID, 1], F32, tag=f"b2_{c}", name=f"b2_{c}")
            b3t[c] = sp.tile([1, 1], F32, tag=f"b3_{c}", name=f"b3_{c}")
            nc.scalar.dma_start(w3t[c][:, :], w3[c])
            nc.scalar.dma_start(b2t[c][:, :], b2[c])
            nc.scalar.dma_start(b3t[c][:, :], b3[c])

        h1t_all = []
        for c in range(C_LOC):
            # stage 1: ps1[bt][b, h] += xT[k, bt].T @ W1[k, :]
            ps1 = [pp.tile([128, H], F32, tag="ps", name=f"ps1_{c}_{bt}")
                   for bt in range(NBT)]
            for kq in range(NKQ):
                w1q = wp.tile([128, KQ * H], F16, tag="w1q",
                              name=f"w1q_{c}_{kq}")
                nc.scalar.dma_start(w1q[:, :], w1p[c, kq])
                for kk in range(KQ):
                    k = kq * KQ + kk
                    xtt = xp.tile([128, B_LOC], F16, tag="xtt",
                                  name=f"xtt_{c}_{k}")
                    nc.sync.dma_start(xtt[:, :], xt[c, k * 128:(k + 1) * 128, :])
                    for bt in range(NBT):
                        nc.tensor.matmul(
                            ps1[bt][:, :],
                            xtt[:, bt * 128:(bt + 1) * 128],
                            w1q[:, kk * H:(kk + 1) * H],
                            start=(k == 0),
                            stop=(k == NK - 1),
                        )

            # epilogue: drain -> PE transpose (to h-major) -> relu+b1
            tps = [pp.tile([128, NBT * 128], F16, tag="ps",
                           name=f"tps_{c}_{i}") for i in range(3)]
            for bt in range(NBT):
                h1b = hp.tile([128, H], F16, tag="h1b", name=f"h1b_{c}_{bt}")
                nc.vector.tensor_copy(h1b[:, :], ps1[bt][:, :])
                for i, (h0, hs) in enumerate(HCH):
                    nc.tensor.transpose(
                        tps[i][:hs, bt * 128:(bt + 1) * 128],
                        h1b[:, h0:h0 + hs],
                        ident[:, :],
                    )
            h1t = [h1p.tile([128, B_LOC], F16, tag="h1t",
                            name=f"h1t_{c}_{i}") for i in range(3)]
            for i, (h0, hs) in enumerate(HCH):
                nc.scalar.activation(
                    h1t[i][:hs, :], tps[i][:hs, :], RELU,
                    bias=b1t[c, i][:hs, :],
                )
            h1t_all.append(h1t)

        # stages 2+3 for all channels (PE work here is ~2% of stage 1)
        for c in range(C_LOC):
            h2t = op.tile([MID, B_LOC], F16, tag="h2t", name=f"h2t_{c}")
            p2 = [pp.tile([MID, 512], F32, tag="ps", name=f"p2_{c}_{j}")
                  for j in range(NJ)]
            for j in range(NJ):
                for i, (h0, hs) in enumerate(HCH):
                    nc.tensor.matmul(
                        p2[j][:, :],
                        w2t[c, i][:hs, :],
                        h1t_all[c][i][:hs, j * 512:(j + 1) * 512],
                        start=(i == 0),
                        stop=(i == len(HCH) - 1),
                    )
                nc.scalar.activation(
                    h2t[:, j * 512:(j + 1) * 512], p2[j][:, :], RELU,
                    bias=b2t[c][:, :],
                )
            p3 = [pp.tile([1, 512], F32, tag="ps", name=f"p3_{c}_{j}")
                  for j in range(NJ)]
            sct = op.tile([1, B_LOC], F32, tag="sct", name=f"sct_{c}")
            for j in range(NJ):
                nc.tensor.matmul(
                    p3[j][:, :], w3t[c][:, :],
                    h2t[:, j * 512:(j + 1) * 512],
                    start=True, stop=True,
                )
                nc.scalar.activation(
                    sct[:, j * 512:(j + 1) * 512], p3[j][:, :], IDENT,
                    bias=b3t[c][:, :],
                )
            nc.sync.dma_start(out[c:c + 1, :], sct[0:1, :])

    nc.compile()
    return nc


_NC_CACHE = {}


def _get_nc():
    if "nc" not in _NC_CACHE:
        _NC_CACHE["nc"] = build_nc()
    return _NC_CACHE["nc"]


def _transpose_shard(xs):
    """[b_loc, c_loc, n] (f16) -> contiguous [c_loc, n, b_loc]."""
    try:
        import torch
        import os
        try:
            torch.set_num_threads(max(os.cpu_count() or 1, 1))
        except Exception:
            pass
        return torch.from_numpy(np.ascontiguousarray(xs)).permute(
            1, 2, 0).contiguous().numpy()
    except ImportError:
        return np.ascontiguousarray(np.transpose(xs, (1, 2, 0)))


def _pack_w1(w1_f32):
    """[c_loc, N, H] f32 -> [c_loc, NKQ, 128, KQ*H] f16 with 4 k-chunks
    interleaved per row so DMA lines are 2288B."""
    w = w1_f32.astype(np.float16)
    w = w.reshape(C_LOC, NKQ, KQ, 128, H).transpose(0, 1, 3, 2, 4)
    return np.ascontiguousarray(w.reshape(C_LOC, NKQ, 128, KQ * H))


def kernel(x, W1, b1, W2, b2, W3, b3, Wf1, bf1, Wf2, bf2):
    x = np.asarray(x, dtype=np.float32)
    W1 = np.asarray(W1, dtype=np.float32)
    b1 = np.asarray(b1, dtype=np.float32)
    W2 = np.asarray(W2, dtype=np.float32)
    b2 = np.asarray(b2, dtype=np.float32)
    W3 = np.asarray(W3, dtype=np.float32)
    b3 = np.asarray(b3, dtype=np.float32)

    nc = _get_nc()

    # cast before transposing so the shuffle moves half the bytes
    x16 = np.ascontiguousarray(x, dtype=np.float16)

    in_maps = []
    for ib in range(BG):
        bs = slice(ib * B_LOC, (ib + 1) * B_LOC)
        for ic in range(CG):
            cs = slice(ic * C_LOC, (ic + 1) * C_LOC)
            in_maps.append({
                "xt": _transpose_shard(x16[bs, cs, :]),
                "w1p": _pack_w1(W1[cs]),
                "b1": np.ascontiguousarray(b1[cs])[:, :, None],
                "w2": np.ascontiguousarray(W2[cs], dtype=np.float16),
                "b2": np.ascontiguousarray(b2[cs])[:, :, None],
                "w3": np.ascontiguousarray(W3[cs], dtype=np.float16)[:, :, None],
                "b3": np.ascontiguousarray(b3[cs])[:, None, None],
            })

    res = run_bass_kernel_spmd(nc, in_maps, list(range(BG * CG)))
    LAST["exec_time_ns"] = res.exec_time_ns
    LAST["results"] = res

    scalars = np.empty((B, C), np.float32)
    idx = 0
    for ib in range(BG):
        bs = slice(ib * B_LOC, (ib + 1) * B_LOC)
        for ic in range(CG):
            cs = slice(ic * C_LOC, (ic + 1) * C_LOC)
            scalars[bs, cs] = res.results[idx]["out"].T
            idx += 1

    # Final tiny MLP (C -> 30 -> lowdim) on host in fp32.
    h = np.maximum(scalars @ np.asarray(Wf1, np.float32)
                   + np.asarray(bf1, np.float32), 0.0)
    return (h @ np.asarray(Wf2, np.float32)
            + np.asarray(bf2, np.float32)).astype(np.float32)


# revision 5
# speedup vs baseline: 2.1127x; 1.6560x over previous
"""Trainium2 Bass kernel for nn_ChannelizedLinearCompression.

Computation (fp32 reference):
    h1      = relu(einsum('bcn,cnh->bch', x, W1) + b1)   # [B, C, H]
    h2      = relu(einsum('bch,chk->bck', h1, W2) + b2)  # [B, C, 10]
    scalars = einsum('bck,ck->bc', h2, W3) + b3          # [B, C]
    out     = relu(scalars @ Wf1 + bf1) @ Wf2 + bf2      # [B, 16]

Sharding: 2 batch groups x 4 channel groups over 8 cores; the tiny final
MLP (0.003% of FLOPs) runs on host.

Stage 1 (99.9% of FLOPs) runs in fp8 e4m3 with DoubleRow perf mode: each
matmul contracts K=256 (two 128-row k-blocks packed per partition in both
operands), streaming 512 output columns — 2x the fp16 PE rate. Downstream
averaging washes the fp8 noise out: measured end-to-end rel err ~7e-4 vs
the 2e-2 gate. h-orientation (h on psum partitions, batch streamed) keeps
each stationary W1 chunk live for 2 matmuls so LDWEIGHTS (256 cols, FWL
off under DoubleRow) hides behind streaming, and stage-1 output lands
h-major so stages 2/3 chain directly and b1 rides the per-partition
ScalarE activation bias.

DMA: x is host-packed [c, kpair, p, r, b] so fp8 pair-tiles stream as
[128, 2048B] lines at full rate (sync HWDGE queue). W1 is host-packed 8
k-blocks wide ([128, 2288B] lines; raw 286B fp8 rows would run at ~1/8
DMA rate) on the scalar HWDGE queue. Small per-channel weights go through
the GpSimd SWDGE queue so they don't clog HWDGE descriptor-gen at kernel
start (a 600ns/DMA cost that previously delayed the first matmul by 26us).
"""

import numpy as np

from contextlib import ExitStack

import concourse.bass as bass
import concourse.tile as tile
from concourse import bacc, mybir
from concourse.bass_utils import run_bass_kernel_spmd
from concourse._compat import get_trn_type

# Problem shapes (hardcoded; kernel.py must be self-contained).
B, C, N = 2048, 12, 8192
H, MID = 286, 10
FINAL_HIDDEN, LOWDIM = 30, 16
BG, CG = 2, 4  # batch groups x channel groups = 8 cores
B_LOC, C_LOC = B // BG, C // CG
NKP = N // 256           # 32 DoubleRow k-pairs (K=256 each)
WQ = 4                   # k-pairs per packed W1 DMA (2288B lines)
NWQ = NKP // WQ          # 8 W1 DMAs per channel
NJ = B_LOC // 512        # 2 psum-width groups
HCH = [(0, 128), (128, 128), (256, 30)]
H_PAD = 288              # DoubleRow LDWEIGHTS needs pair-dim step % 16 == 0

F8 = mybir.dt.float8e4
F16 = mybir.dt.float16
F32 = mybir.dt.float32
DR = mybir.MatmulPerfMode.DoubleRow
RELU = mybir.ActivationFunctionType.Relu
IDENT = mybir.ActivationFunctionType.Identity

LAST = {}  # introspection for test.py (exec_time_ns etc.); harness ignores


def build_nc():
    nc = bacc.Bacc(get_trn_type() or "TRN2", target_bir_lowering=False)
    xt8 = nc.declare_dram_parameter("xt8", [C_LOC, NKP, 128, 2, B_LOC], F8,
                                    isOutput=False)
    w1p = nc.declare_dram_parameter("w1p", [C_LOC, NWQ, 128, WQ, 2, H_PAD],
                                    F8, isOutput=False)
    b1 = nc.declare_dram_parameter("b1", [C_LOC, H, 1], F32, isOutput=False)
    w2 = nc.declare_dram_parameter("w2", [C_LOC, H, MID], F16, isOutput=False)
    b2 = nc.declare_dram_parameter("b2", [C_LOC, MID, 1], F32, isOutput=False)
    w3 = nc.declare_dram_parameter("w3", [C_LOC, MID, 1], F16, isOutput=False)
    b3 = nc.declare_dram_parameter("b3", [C_LOC, 1, 1], F32, isOutput=False)
    out = nc.declare_dram_parameter("out", [C_LOC, B_LOC], F32, isOutput=True)

    with tile.TileContext(nc) as tc, ExitStack() as ctx:
        xp = ctx.enter_context(tc.tile_pool(name="xp", bufs=8))
        wp = ctx.enter_context(tc.tile_pool(name="wp", bufs=3))
        h1p = ctx.enter_context(tc.tile_pool(name="h1p", bufs=3 * C_LOC))
        sp = ctx.enter_context(tc.tile_pool(name="sp", bufs=1))
        op = ctx.enter_context(tc.tile_pool(name="op", bufs=3))
        pp = ctx.enter_context(
            tc.tile_pool(name="pp", bufs=8, space=bass.MemorySpace.PSUM)
        )

        # Small per-channel weights via the GpSimd software DGE so the two
        # HWDGE queues start on the big streams immediately.
        b1t, w2t, w3t, b2t, b3t = {}, {}, {}, {}, {}
        for c in range(C_LOC):
            for i, (h0, hs) in enumerate(HCH):
                b1t[c, i] = sp.tile([128, 1], F32, tag=f"b1_{c}_{i}",
                                    name=f"b1_{c}_{i}")
                w2t[c, i] = sp.tile([128, MID], F16, tag=f"w2_{c}_{i}",
                                    name=f"w2_{c}_{i}")
                nc.gpsimd.dma_start(b1t[c, i][:hs, :], b1[c, h0:h0 + hs, :])
                nc.gpsimd.dma_start(w2t[c, i][:hs, :], w2[c, h0:h0 + hs, :])
            w3t[c] = sp.tile([MID, 1], F16, tag=f"w3_{c}", name=f"w3_{c}")
            b2t[c] = sp.tile([MID, 1], F32, tag=f"b2_{c}", name=f"b2_{c}")
            b3t[c] = sp.tile([1, 1], F32, tag=f"b3_{c}", name=f"b3_{c}")
            nc.gpsimd.dma_start(w3t[c][:, :], w3[c])
            nc.gpsimd.dma_start(b2t[c][:, :], b2[c])
            nc.gpsimd.dma_start(b3t[c][:, :], b3[c])

        h1t_all = []
        for c in range(C_LOC):
            # stage 1: ps[i][j][h, b] += W1pair[k, :, h].T2 @ xTpair[k, :, b]
            ps = [[pp.tile([128, 512], F32, tag="ps", name=f"ps_{c}_{i}_{j}")
                   for j in range(NJ)] for i in range(len(HCH))]
            for kp in range(NKP):
                if kp % WQ == 0:
                    w1q = wp.tile([128, WQ, 2, H_PAD], F8, tag="w1q",
                                  name=f"w1q_{c}_{kp // WQ}")
                    nc.scalar.dma_start(w1q[:, :, :, :], w1p[c, kp // WQ])
                xtt = xp.tile([128, 2, B_LOC], F8, tag="xtt",
                              name=f"xtt_{c}_{kp}")
                nc.sync.dma_start(xtt[:, :, :], xt8[c, kp])
                for i, (h0, hs) in enumerate(HCH):
                    for j in range(NJ):
                        nc.tensor.matmul(
                            ps[i][j][:hs, :],
                            w1q[:, kp % WQ, :, h0:h0 + hs],
                            xtt[:, :, j * 512:(j + 1) * 512],
                            start=(kp == 0),
                            stop=(kp == NKP - 1),
                            perf_mode=DR,
                        )

            h1t = [h1p.tile([128, B_LOC], F16, tag="h1t",
                            name=f"h1t_{c}_{i}") for i in range(len(HCH))]
            for i, (h0, hs) in enumerate(HCH):
                for j in range(NJ):
                    nc.scalar.activation(
                        h1t[i][:hs, j * 512:(j + 1) * 512],
                        ps[i][j][:hs, :], RELU,
                        bias=b1t[c, i][:hs, :],
                    )
            h1t_all.append(h1t)

        # stages 2+3 for all channels (~2% of stage-1 PE work)
        for c in range(C_LOC):
            h2t = op.tile([MID, B_LOC], F16, tag="h2t", name=f"h2t_{c}")
            p2 = [pp.tile([MID, 512], F32, tag="ps", name=f"p2_{c}_{j}")
                  for j in range(NJ)]
            for j in range(NJ):
                for i, (h0, hs) in enumerate(HCH):
                    nc.tensor.matmul(
                        p2[j][:, :],
                        w2t[c, i][:hs, :],
                        h1t_all[c][i][:hs, j * 512:(j + 1) * 512],
                        start=(i == 0),
                        stop=(i == len(HCH) - 1),
                    )
                nc.scalar.activation(
                    h2t[:, j * 512:(j + 1) * 512], p2[j][:, :], RELU,
                    bias=b2t[c][:, :],
                )
            p3 = [pp.tile([1, 512], F32, tag="ps", name=f"p3_{c}_{j}")
                  for j in range(NJ)]
            sct = op.tile([1, B_LOC], F32, tag="sct", name=f"sct_{c}")
            for j in range(NJ):
                nc.tensor.matmul(
                    p3[j][:, :], w3t[c][:, :],
                    h2t[:, j * 512:(j + 1) * 512],
                    start=True, stop=True,
                )
                nc.scalar.activation(
                    sct[:, j * 512:(j + 1) * 512], p3[j][:, :], IDENT,
                    bias=b3t[c][:, :],
                )
            nc.sync.dma_start(out[c:c + 1, :], sct[0:1, :])

    nc.compile()
    return nc


_NC_CACHE = {}


def _get_nc():
    if "nc" not in _NC_CACHE:
        _NC_CACHE["nc"] = build_nc()
    return _NC_CACHE["nc"]


def _f8(arr):
    import ml_dtypes
    return arr.astype(ml_dtypes.float8_e4m3)


def _pack_x(x8_shard):
    """[b_loc, c_loc, N] fp8 -> [c_loc, NKP, 128, 2, b_loc]: k-pair tiles
    with the two 128-row k-blocks interleaved per partition (2KB lines)."""
    a = np.ascontiguousarray(x8_shard.transpose(1, 2, 0))  # [c, N, b]
    a = a.reshape(C_LOC, NKP, 2, 128, B_LOC).transpose(0, 1, 3, 2, 4)
    return np.ascontiguousarray(a)


def _pack_w1(w1_f32):
    """[c_loc, N, H] f32 -> [c_loc, NWQ, 128, WQ, 2, H_PAD] fp8 (2304B
    lines; H padded to 288 for the DoubleRow step%16 ISA rule)."""
    w = np.zeros((C_LOC, N, H_PAD), np.float32)
    w[:, :, :H] = w1_f32
    w = _f8(w)
    w = w.reshape(C_LOC, NWQ, WQ, 2, 128, H_PAD).transpose(0, 1, 4, 2, 3, 5)
    return np.ascontiguousarray(w)


def kernel(x, W1, b1, W2, b2, W3, b3, Wf1, bf1, Wf2, bf2):
    x = np.asarray(x, dtype=np.float32)
    W1 = np.asarray(W1, dtype=np.float32)
    b1 = np.asarray(b1, dtype=np.float32)
    W2 = np.asarray(W2, dtype=np.float32)
    b2 = np.asarray(b2, dtype=np.float32)
    W3 = np.asarray(W3, dtype=np.float32)
    b3 = np.asarray(b3, dtype=np.float32)

    nc = _get_nc()

    x8 = _f8(x)  # cast before the shuffle so it moves 1/4 the bytes

    in_maps = []
    for ib in range(BG):
        bs = slice(ib * B_LOC, (ib + 1) * B_LOC)
        for ic in range(CG):
            cs = slice(ic * C_LOC, (ic + 1) * C_LOC)
            in_maps.append({
                "xt8": _pack_x(x8[bs, cs, :]),
                "w1p": _pack_w1(W1[cs]),
                "b1": np.ascontiguousarray(b1[cs])[:, :, None],
                "w2": np.ascontiguousarray(W2[cs], dtype=np.float16),
                "b2": np.ascontiguousarray(b2[cs])[:, :, None],
                "w3": np.ascontiguousarray(W3[cs], dtype=np.float16)[:, :, None],
                "b3": np.ascontiguousarray(b3[cs])[:, None, None],
            })

    res = run_bass_kernel_spmd(nc, in_maps, list(range(BG * CG)))
    LAST["exec_time_ns"] = res.exec_time_ns
    LAST["results"] = res

    scalars = np.empty((B, C), np.float32)
    idx = 0
    for ib in range(BG):
        bs = slice(ib * B_LOC, (ib + 1) * B_LOC)
        for ic in range(CG):
            cs = slice(ic * C_LOC, (ic + 1) * C_LOC)
            scalars[bs, cs] = res.results[idx]["out"].T
            idx += 1

    # Final tiny MLP (C -> 30 -> lowdim) on host in fp32.
    h = np.maximum(scalars @ np.asarray(Wf1, np.float32)
                   + np.asarray(bf1, np.float32), 0.0)
    return (h @ np.asarray(Wf2, np.float32)
            + np.asarray(bf2, np.float32)).astype(np.float32)
